# revision 9
# baseline (speedup 1.0000x reference)
"""Strided depthwise-conv ("CompressKV") kernel for 8 Trainium2 NeuronCores.

y[b,m,h,d] = (sum_k x[b, 16*m+k, h, d] * w[k] + sum_k pe[k,d]*w[k]) / 32
B=4, N=16384, H=8, D=128, K=32, STRIDE=16, M=1023.

Strategy
--------
Shard: core <-> (batch b, sequence half). Each core owns one contiguous
token slab x[b, 8192*hh : 8192*hh+8320] (zero-padded past N), all 8 heads.

Compute: the strided conv is expressed as 17 banded-weight matmuls per
128-m output tile on the TensorEngine:

    y[m', f] = sum_i  W_i[n', m'].T @ X_chunk[16*ot+i][n', f]

where chunk g = tokens [128g, 128g+128), f = (head, d) flattened (1024),
W_i[n', m'] = w[128*i + n' - 16*m'] / 32  (zero outside [0,32)).
W_i is built on the host from `weight` and fed as a small extra input.
The pe-bias vector enters the same PSUM accumulation as a rank-2 matmul
(ones.T @ [bias_hi; bias_lo], hi/lo bf16 split keeps it fp32-exact).
x is cast to bf16 on the host (halves DMA bytes; matmul runs at bf16
rate).  PSUM bank limit (512 fp32) => each logical matmul is two
512-wide matmuls.  Eviction is a scalar-engine copy + scalar-issued
store so no DMA instruction ever needs more than one semaphore wait
(walrus DIRECT2D limit).
"""

import numpy as np
import ml_dtypes
from contextlib import ExitStack

import concourse.bass as bass
import concourse.mybir as mybir
import concourse.tile as tile
from concourse.bass import ds, ts
from concourse.bass_utils import run_bass_kernel_spmd

BF16 = ml_dtypes.bfloat16


class _SplitDrainTileContext(tile.TileContext):
    """TileContext whose kernel-tail drain carries at most one sem wait.

    TRN2 instructions have a single sync-wait slot; the stock tail drain
    aggregates one wait per logical processor (14 here), which walrus
    rejects.  Move the extras onto dedicated single-wait nops on the same
    (sync) queue ahead of the all-engine barrier — identical semantics.
    """

    def _drain_and_barrier(self, tick_clock, wait_clock):
        import bass_rust
        from concourse.vector_clock import ScopedClock

        drain_inst = self.nc.sync.drain()
        wait_clock.add_sem_waits(
            drain_inst.ins, ScopedClock({None: tick_clock.global_clock}))
        si = drain_inst.ins.sync_info
        if si is not None and len(si.on_wait) > 1:
            waits = list(si.on_wait)
            drain_inst.ins.sync_info = bass_rust.SyncInfo(
                on_wait=[waits[0]], on_update=list(si.on_update))
            for w in waits[1:]:
                nop = self.nc.sync.nop(hint="drain_split", nofuse=True)
                nop.ins.sync_info = bass_rust.SyncInfo(
                    on_wait=[w], on_update=[])

        self.nc.all_engine_barrier()
        assert self.sems is not None
        popped = self.nc._tile_sem_poison_stack.pop()
        assert popped is self._sem_poison
        self.nc.clear_and_free_semaphores(
            list(self.sems.allocated().values()))
        self.nc.all_engine_barrier()

B, N, H, D = 4, 16384, 8, 128
KS, STRIDE = 32, 16
M = (N - KS) // STRIDE + 1      # 1023
NCORES = 8
F = H * D                        # 1024 free elems (head, d)
P = 128                          # partitions / tokens per chunk
NW = 17                          # band matrices per output tile
CH = 65                          # chunks per core slab (8320 tokens)
OT = 4                           # output tiles of 128 m per core
T_SLAB = CH * P                  # 8320
HF = F // 2                      # 512 = one PSUM bank of fp32

_prog_cache = {}


def _build_program():
    """Build the SPMD Bass/Tile program (identical for all 8 cores)."""
    nc = bass.Bass("TRN2", target_bir_lowering=False, debug=False,
                   num_devices=NCORES)
    x_d = nc.dram_tensor("x", [T_SLAB, F], mybir.dt.bfloat16,
                         kind="ExternalInput").ap()
    w_d = nc.dram_tensor("wt", [P, NW * P], mybir.dt.bfloat16,
                         kind="ExternalInput").ap()
    br_d = nc.dram_tensor("brow", [2, F], mybir.dt.bfloat16,
                          kind="ExternalInput").ap()
    on_d = nc.dram_tensor("ones2", [2, P], mybir.dt.bfloat16,
                          kind="ExternalInput").ap()
    y_d = nc.dram_tensor("y", [OT * P, F], mybir.dt.float32,
                         kind="ExternalOutput").ap()

    with _SplitDrainTileContext(nc) as tc, ExitStack() as ctx:
        const_pool = ctx.enter_context(tc.tile_pool(name="const", bufs=1))
        chunk_pool = ctx.enter_context(
            tc.tile_pool(name="chunks", bufs=CH + 1))
        out_pool = ctx.enter_context(tc.tile_pool(name="out", bufs=OT))
        psum_pool = ctx.enter_context(
            tc.tile_pool(name="psum", bufs=OT, space="PSUM"))

        wt = const_pool.tile([P, NW * P], mybir.dt.bfloat16)
        nc.scalar.dma_start(out=wt[:], in_=w_d)
        brow = const_pool.tile([2, F], mybir.dt.bfloat16)
        nc.scalar.dma_start(out=brow[:], in_=br_d)
        ones2 = const_pool.tile([2, P], mybir.dt.bfloat16)
        nc.scalar.dma_start(out=ones2[:], in_=on_d)

        psum_tiles = {}

        def evict(ot):
            o = out_pool.tile([P, F], mybir.dt.float32)
            nc.vector.tensor_copy(o[:], psum_tiles[ot][:])
            nc.gpsimd.dma_start(out=y_d[ds(ot * P, P)], in_=o[:])

        for g in range(CH):
            chunk = chunk_pool.tile([P, F], mybir.dt.bfloat16)
            nc.sync.dma_start(out=chunk[:], in_=x_d[ds(g * P, P)])
            ot, i = g // 16, g % 16
            if i == 0 and g > 0:
                # W_16 closes the previous output tile's accumulation
                for hf in range(2):
                    nc.tensor.matmul(
                        psum_tiles[ot - 1][:, ts(hf, HF)],
                        lhsT=wt[:, ts(16, P)],
                        rhs=chunk[:, ts(hf, HF)],
                        start=False, stop=True)
                evict(ot - 1)
            if g < 16 * OT:
                if i == 0:
                    psum_tiles[ot] = psum_pool.tile([P, F], mybir.dt.float32,
                                                    name="ps", tag="ps")
                    # bias enters the accumulation as a rank-2 matmul
                    for hf in range(2):
                        nc.tensor.matmul(
                            psum_tiles[ot][:, ts(hf, HF)],
                            lhsT=ones2[:],
                            rhs=brow[:, ts(hf, HF)],
                            start=True, stop=False)
                for hf in range(2):
                    nc.tensor.matmul(
                        psum_tiles[ot][:, ts(hf, HF)],
                        lhsT=wt[:, ts(i, P)],
                        rhs=chunk[:, ts(hf, HF)],
                        start=False, stop=False)
    return nc


def _get_program():
    if "nc" not in _prog_cache:
        _prog_cache["nc"] = _build_program()
    return _prog_cache["nc"]


def _host_prep(x, weight, pe):
    """Build per-core input maps (band matrices, bias rows, bf16 slabs)."""
    x = np.asarray(x)
    weight = np.asarray(weight, dtype=np.float32)
    pe = np.asarray(pe, dtype=np.float32)

    i_ = np.arange(NW)[:, None, None]
    n_ = np.arange(P)[None, :, None]
    m_ = np.arange(P)[None, None, :]
    k_ = 128 * i_ + n_ - 16 * m_
    wt = np.where((k_ >= 0) & (k_ < KS),
                  weight[np.clip(k_, 0, KS - 1)] / KS, 0.0)
    # [NW, n, m] -> [n, NW*m] so the SBUF tile loads with one plain 2D DMA
    wt = wt.astype(BF16).transpose(1, 0, 2).reshape(P, NW * P)

    bias_d = ((weight[:, None].astype(np.float64) * pe).sum(0) / KS
              ).astype(np.float32)
    bias_hi = bias_d.astype(BF16)
    bias_lo = (bias_d - bias_hi.astype(np.float32)).astype(BF16)
    brow = np.stack([np.tile(bias_hi, H), np.tile(bias_lo, H)])  # [2, 1024]
    ones2 = np.ones((2, P), dtype=BF16)

    in_maps = []
    for c in range(NCORES):
        b, hh = c // 2, c % 2
        base = 8192 * hh
        t_valid = min(N - base, T_SLAB)
        slab = np.zeros((T_SLAB, F), dtype=BF16)
        slab[:t_valid] = x[b, base:base + t_valid].reshape(t_valid, F)
        in_maps.append({"x": slab, "wt": wt, "brow": brow, "ones2": ones2})
    return in_maps


def _assemble(results, dtype):
    y = np.empty((B, M, H, D), dtype=np.float32)
    for c in range(NCORES):
        b, hh = c // 2, c % 2
        rows = 512 if hh == 0 else M - 512
        part = results[c]["y"].reshape(OT * P, H, D)
        y[b, 512 * hh:512 * hh + rows] = part[:rows]
    return y.astype(dtype, copy=False)


def kernel(x, weight, pe):
    nc = _get_program()
    in_maps = _host_prep(x, weight, pe)
    res = run_bass_kernel_spmd(nc, in_maps, list(range(NCORES)))
    return _assemble(res.results, np.asarray(x).dtype)


# revision 12
# speedup vs baseline: 2894.9863x; 2894.9863x over previous
"""Strided depthwise-conv ("CompressKV") kernel for 8 Trainium2 NeuronCores.

y[b,m,h,d] = (sum_k x[b, 16*m+k, h, d] * w[k] + sum_k pe[k,d]*w[k]) / 32
B=4, N=16384, H=8, D=128, K=32, STRIDE=16, M=1023.

Strategy
--------
Shard: core <-> (batch b, sequence half). Each core owns one contiguous
token slab x[b, 8192*hh : 8192*hh+8320] (zero-padded past N), all 8 heads.

Compute: the strided conv is expressed as 17 banded-weight matmuls per
128-m output tile on the TensorEngine:

    y[m', f] = sum_i  W_i[n', m'].T @ X_chunk[16*ot+i][n', f]

where chunk g = tokens [128g, 128g+128), f = (head, d) flattened (1024),
W_i[n', m'] = w[128*i + n' - 16*m'] / 32  (zero outside [0,32)).
W_i is built on the host from `weight` and fed as a small extra input.
The pe-bias vector enters the same PSUM accumulation as a rank-2 matmul
(ones.T @ [bias_hi; bias_lo], hi/lo bf16 split keeps it fp32-exact).
x is cast to bf16 on the host (halves DMA bytes; matmul runs at bf16
rate).  PSUM bank limit (512 fp32) => each logical matmul is two
512-wide matmuls.  Eviction is a scalar-engine copy + scalar-issued
store so no DMA instruction ever needs more than one semaphore wait
(walrus DIRECT2D limit).
"""

import numpy as np
import ml_dtypes
from contextlib import ExitStack

import concourse.bass as bass
import concourse.mybir as mybir
import concourse.tile as tile
from concourse.bass import ds, ts
from concourse.bass_utils import run_bass_kernel_spmd

BF16 = ml_dtypes.bfloat16


class _SplitDrainTileContext(tile.TileContext):
    """TileContext whose kernel-tail drain carries at most one sem wait.

    TRN2 instructions have a single sync-wait slot; the stock tail drain
    aggregates one wait per logical processor (14 here), which walrus
    rejects.  Move the extras onto dedicated single-wait nops on the same
    (sync) queue ahead of the all-engine barrier — identical semantics.
    """

    def _drain_and_barrier(self, tick_clock, wait_clock):
        import bass_rust
        from concourse.vector_clock import ScopedClock

        drain_inst = self.nc.sync.drain()
        wait_clock.add_sem_waits(
            drain_inst.ins, ScopedClock({None: tick_clock.global_clock}))
        si = drain_inst.ins.sync_info
        if si is not None and len(si.on_wait) > 1:
            waits = list(si.on_wait)
            drain_inst.ins.sync_info = bass_rust.SyncInfo(
                on_wait=[waits[0]], on_update=list(si.on_update))
            for w in waits[1:]:
                nop = self.nc.sync.nop(hint="drain_split", nofuse=True)
                nop.ins.sync_info = bass_rust.SyncInfo(
                    on_wait=[w], on_update=[])

        self.nc.all_engine_barrier()
        assert self.sems is not None
        popped = self.nc._tile_sem_poison_stack.pop()
        assert popped is self._sem_poison
        self.nc.clear_and_free_semaphores(
            list(self.sems.allocated().values()))
        self.nc.all_engine_barrier()

B, N, H, D = 4, 16384, 8, 128
KS, STRIDE = 32, 16
M = (N - KS) // STRIDE + 1      # 1023
NCORES = 8
F = H * D                        # 1024 free elems (head, d)
P = 128                          # partitions / tokens per chunk
NW = 17                          # band matrices per output tile
CH = 65                          # chunks per core slab (8320 tokens)
OT = 4                           # output tiles of 128 m per core
T_SLAB = CH * P                  # 8320
HF = F // 2                      # 512 = one PSUM bank of fp32

_prog_cache = {}


def _split_multi_waits(nc):
    """TRN2 instructions carry one sync-wait slot; Tile sometimes attaches
    more (slot-recycle + DMA-lane).  Hoist extras onto single-wait nops
    inserted just before the instruction on the same engine queue —
    identical semantics, accepted by walrus codegen."""
    import bass_rust
    for func in nc.m.functions:
        for bb in func.blocks:
            insts = list(bb.instructions)
            out, changed = [], False
            for inst in insts:
                si = inst.sync_info
                if si is not None and len(si.on_wait) > 1:
                    waits = list(si.on_wait)
                    for k, w in enumerate(waits[:-1]):
                        nop = mybir.InstNoOp(name=f"{inst.name}-ws{k}")
                        nop.engine = inst.engine
                        nop.sync_info = bass_rust.SyncInfo(
                            on_wait=[w], on_update=[])
                        out.append(nop)
                    inst.sync_info = bass_rust.SyncInfo(
                        on_wait=[waits[-1]], on_update=list(si.on_update))
                    changed = True
                out.append(inst)
            if changed:
                bb.instructions = out


def _build_program(reps=1):
    """Build the SPMD Bass/Tile program (identical for all 8 cores).

    reps>1 repeats the whole pipeline inside one NEFF (benchmark use:
    slope of wall time vs reps isolates device execution time from the
    dispatch round trip)."""
    nc = bass.Bass("TRN2", target_bir_lowering=False, debug=False,
                   num_devices=NCORES)
    x_d = nc.dram_tensor("x", [T_SLAB, F], mybir.dt.bfloat16,
                         kind="ExternalInput").ap()
    w_d = nc.dram_tensor("wt", [P, NW * P], mybir.dt.bfloat16,
                         kind="ExternalInput").ap()
    br_d = nc.dram_tensor("brow", [2, F], mybir.dt.bfloat16,
                          kind="ExternalInput").ap()
    on_d = nc.dram_tensor("ones2", [2, P], mybir.dt.bfloat16,
                          kind="ExternalInput").ap()
    y_d = nc.dram_tensor("y", [OT * P, F], mybir.dt.float32,
                         kind="ExternalOutput").ap()

    with _SplitDrainTileContext(nc) as tc, ExitStack() as ctx:
        const_pool = ctx.enter_context(tc.tile_pool(name="const", bufs=1))
        chunk_pool = ctx.enter_context(
            tc.tile_pool(name="chunks", bufs=CH + 1))
        out_pool = ctx.enter_context(tc.tile_pool(name="out", bufs=OT))
        psum_pool = ctx.enter_context(
            tc.tile_pool(name="psum", bufs=OT, space="PSUM"))

        wt = const_pool.tile([P, NW * P], mybir.dt.bfloat16)
        nc.scalar.dma_start(out=wt[:], in_=w_d)
        brow = const_pool.tile([2, F], mybir.dt.bfloat16)
        nc.scalar.dma_start(out=brow[:], in_=br_d)
        ones2 = const_pool.tile([2, P], mybir.dt.bfloat16)
        nc.scalar.dma_start(out=ones2[:], in_=on_d)

        for _rep in range(reps):
            psum_tiles = {}

            def evict(ot):
                o = out_pool.tile([P, F], mybir.dt.float32, name="o", tag="o")
                nc.vector.tensor_copy(o[:], psum_tiles[ot][:])
                nc.gpsimd.dma_start(out=y_d[ds(ot * P, P)], in_=o[:])

            for g in range(CH):
                chunk = chunk_pool.tile([P, F], mybir.dt.bfloat16,
                                        name="chunk", tag="chunk")
                nc.sync.dma_start(out=chunk[:], in_=x_d[ds(g * P, P)])
                ot, i = g // 16, g % 16
                if i == 0 and g > 0:
                    # W_16 closes the previous output tile's accumulation
                    for hf in range(2):
                        nc.tensor.matmul(
                            psum_tiles[ot - 1][:, ts(hf, HF)],
                            lhsT=wt[:, ts(16, P)],
                            rhs=chunk[:, ts(hf, HF)],
                            start=False, stop=True)
                    evict(ot - 1)
                if g < 16 * OT:
                    if i == 0:
                        psum_tiles[ot] = psum_pool.tile(
                            [P, F], mybir.dt.float32, name="ps", tag="ps")
                        # bias enters the accumulation as a rank-2 matmul
                        for hf in range(2):
                            nc.tensor.matmul(
                                psum_tiles[ot][:, ts(hf, HF)],
                                lhsT=ones2[:],
                                rhs=brow[:, ts(hf, HF)],
                                start=True, stop=False)
                    for hf in range(2):
                        nc.tensor.matmul(
                            psum_tiles[ot][:, ts(hf, HF)],
                            lhsT=wt[:, ts(i, P)],
                            rhs=chunk[:, ts(hf, HF)],
                            start=False, stop=False)
    _split_multi_waits(nc)
    return nc


def _get_program(reps=1):
    if reps not in _prog_cache:
        _prog_cache[reps] = _build_program(reps)
    return _prog_cache[reps]


def _host_prep(x, weight, pe):
    """Build per-core input maps (band matrices, bias rows, bf16 slabs)."""
    x = np.asarray(x)
    weight = np.asarray(weight, dtype=np.float32)
    pe = np.asarray(pe, dtype=np.float32)

    i_ = np.arange(NW)[:, None, None]
    n_ = np.arange(P)[None, :, None]
    m_ = np.arange(P)[None, None, :]
    k_ = 128 * i_ + n_ - 16 * m_
    wt = np.where((k_ >= 0) & (k_ < KS),
                  weight[np.clip(k_, 0, KS - 1)] / KS, 0.0)
    # [NW, n, m] -> [n, NW*m] so the SBUF tile loads with one plain 2D DMA
    wt = wt.astype(BF16).transpose(1, 0, 2).reshape(P, NW * P)

    bias_d = ((weight[:, None].astype(np.float64) * pe).sum(0) / KS
              ).astype(np.float32)
    bias_hi = bias_d.astype(BF16)
    bias_lo = (bias_d - bias_hi.astype(np.float32)).astype(BF16)
    brow = np.stack([np.tile(bias_hi, H), np.tile(bias_lo, H)])  # [2, 1024]
    ones2 = np.ones((2, P), dtype=BF16)

    in_maps = []
    for c in range(NCORES):
        b, hh = c // 2, c % 2
        base = 8192 * hh
        t_valid = min(N - base, T_SLAB)
        slab = np.zeros((T_SLAB, F), dtype=BF16)
        slab[:t_valid] = x[b, base:base + t_valid].reshape(t_valid, F)
        in_maps.append({"x": slab, "wt": wt, "brow": brow, "ones2": ones2})
    return in_maps


def _assemble(results, dtype):
    y = np.empty((B, M, H, D), dtype=np.float32)
    for c in range(NCORES):
        b, hh = c // 2, c % 2
        rows = 512 if hh == 0 else M - 512
        part = results[c]["y"].reshape(OT * P, H, D)
        y[b, 512 * hh:512 * hh + rows] = part[:rows]
    return y.astype(dtype, copy=False)


def kernel(x, weight, pe):
    nc = _get_program()
    in_maps = _host_prep(x, weight, pe)
    res = run_bass_kernel_spmd(nc, in_maps, list(range(NCORES)))
    return _assemble(res.results, np.asarray(x).dtype)
